# revision 17
# baseline (speedup 1.0000x reference)
"""BitNet 3-layer MLP (B=131072, D=256) on 8 TRN2 NeuronCores, data-parallel.

v4 design. Numerics identical to the f32 reference path (rel ~4.5e-3):
x consumed at f32, inter-layer activations exact int16.

Schedule (all phases pipelined per group of 8 row-tiles):
  L0-A   x DMA (half stashed f32, half re-read later) -> BNStats + rowmax on
         DVE, rowmin via gpsimd tensor_tensor min-tree (DVE finishes).
  gamma  per-row math -> [P,1] -> gpsimd partition_all_reduce -> 32B
         AllGather (warmed up by a dummy AllGather at t=0) -> s1/tp.
  sweep  quantize u16 = x*s1 + tp + 1536 (rounds to int in f16; tiles split
         gpsimd/ACT/DVE) -> OFF subtract (DVE 4x f16) -> xbar transpose
         (alternating sync/scalar HWDGE rings to halve the FIFO serial cost)
         -> 2 matmuls per tile (PE, fp16 exact) -> epilogue per half-group
         (ACT relu -> arena i16) -> fused next-layer stats: BNStats (DVE) +
         rowmax (even groups DVE reduce, odd groups gpsimd min... max-tree).
  L2     epilogue scales by beta*gamma/127 (ACT) and DMAs out.

Known-bad paths avoided: bass_isa tensor_tensor_reduce crashes the runtime;
f16 x or f16 h costs ~1e-2 rel err (gamma is extremely sensitive to the
f16 rounding of the argmax element).
"""
import os
import numpy as np
from contextlib import ExitStack

from concourse import bass, tile, mybir
from concourse import bacc
from concourse.bass_utils import run_bass_kernel_spmd
from concourse import bass_isa

P = 128
D = 256
NCORES = 8
B = 131072
B_LOC = B // NCORES          # 16384
T = B_LOC // P               # 128 tiles
G = 8                        # tiles per group
NGRP = T // G                # 16 groups
NSTASH = 6                   # groups of x kept resident in f32
OFF = 1536.0                 # fp16 rounding offset
LN_EPS = 1e-5
QB = 127.0

f32 = mybir.dt.float32
f16 = mybir.dt.float16
i16 = mybir.dt.int16
Alu = mybir.AluOpType
Act = mybir.ActivationFunctionType

NOWARM = os.environ.get("KNOWARM") == "1"
# per-group quantize engine split: tiles [0,a) gpsimd, [a,b) ACT, [b,8) DVE
QSPLIT = [int(c) for c in os.environ.get("KQSPLIT", "44")]


def build_nc():
    nc = bacc.Bacc("TRN2", target_bir_lowering=False, debug=False,
                   num_devices=NCORES)

    x_d = nc.dram_tensor("x", [B_LOC, D], f32, kind="ExternalInput")
    w_d = [nc.dram_tensor(f"W{i+1}", [D, D], f32, kind="ExternalInput")
           for i in range(3)]
    out_d = nc.dram_tensor("out", [B_LOC, D], f32, kind="ExternalOutput")

    with tile.TileContext(nc) as tc:
        with ExitStack() as ctx:
            wt = ctx.enter_context(tc.tile_pool(name="wt", bufs=1))
            stats = ctx.enter_context(tc.tile_pool(name="stats", bufs=2))
            trp = ctx.enter_context(tc.tile_pool(name="trp", bufs=2))
            f32stage = ctx.enter_context(tc.tile_pool(name="f32stage", bufs=3))
            u16p = ctx.enter_context(tc.tile_pool(name="u16p", bufs=2))
            u16tp = ctx.enter_context(tc.tile_pool(name="u16tp", bufs=2))
            smallp = ctx.enter_context(tc.tile_pool(name="smallp", bufs=2))
            psum = ctx.enter_context(tc.tile_pool(name="psum", bufs=4,
                                                  space="PSUM"))
            dram = ctx.enter_context(tc.tile_pool(name="dram", bufs=2,
                                                  space="DRAM"))

            arena = wt.tile([P, T, D], i16)
            xhalf = wt.tile([P, NSTASH * G, D], f32)

            # ---------------- constants ----------------
            onesf = wt.tile([P, 1], f32)
            nc.vector.memset(onesf[:], 1.0)
            repl = wt.tile([1, P], f32)
            nc.vector.memset(repl[:], 1.0)
            epst = wt.tile([P, 1], f32)
            nc.vector.memset(epst[:], LN_EPS)

            # ---------------- warmup AllGather ----------------
            if not NOWARM:
                wsnd_sb = smallp.tile([1, 8], f32, tag="wsnd_sb")
                nc.gpsimd.memset(wsnd_sb[:], 1.0)
                wsnd = dram.tile([1, 8], f32, tag="snd")
                wrcv = dram.tile([1, 8 * NCORES], f32, tag="rcv")
                nc.sync.dma_start(out=wsnd[:], in_=wsnd_sb[:])
                nc.gpsimd.collective_compute(
                    "AllGather", Alu.bypass, ins=[wsnd[:].opt()],
                    outs=[wrcv[:].opt()],
                    replica_groups=[list(range(NCORES))])
                wg64 = smallp.tile([1, 8 * NCORES], f32, tag="wg64")
                nc.sync.dma_start(out=wg64[:], in_=wrcv[:])

            # ---------------- weight prep ----------------
            wqT = []     # [128, 2, 256] fp16: wqT[k_in_band, band, j]
            beta = []    # [1, 1] f32
            for li in range(3):
                wf = wt.tile([P, 2, D], f32, tag="wf")
                nc.sync.dma_start(out=wf[:], in_=w_d[li][:].rearrange(
                    "(a p) d -> p a d", p=P))
                rs = wt.tile([P, 2], f32, tag="rs")
                nc.vector.tensor_reduce(out=rs[:], in_=wf[:],
                                        axis=mybir.AxisListType.X, op=Alu.add)
                rv = wt.tile([P, 1], f32, tag="rv")
                nc.vector.tensor_tensor(out=rv[:], in0=rs[:, 0:1],
                                        in1=rs[:, 1:2], op=Alu.add)
                aps = psum.tile([2, P], f32, tag="mm_ps")
                nc.tensor.matmul(aps[0:1, 0:1], lhsT=onesf[:], rhs=rv[:],
                                 start=True, stop=True)
                alpha = wt.tile([1, 1], f32, tag="alpha")
                nc.scalar.activation(out=alpha[:], in_=aps[0:1, 0:1],
                                     func=Act.Copy, scale=1.0 / (D * D))
                abc_ps = psum.tile([P, 1], f32, tag="mm_ps")
                nc.tensor.matmul(abc_ps[:], lhsT=repl[:], rhs=alpha[:],
                                 start=True, stop=True)
                abc = wt.tile([P, 1], f32, tag="abc")
                nc.vector.tensor_copy(out=abc[:], in_=abc_ps[:])
                wc = wt.tile([P, 2, D], f32, tag="wc")
                nc.vector.tensor_scalar(out=wc[:], in0=wf[:], scalar1=abc[:],
                                        scalar2=None, op0=Alu.subtract)
                ba = wt.tile([P, 2], f32, tag="ba")
                nc.vector.tensor_reduce(out=ba[:], in_=wc[:],
                                        axis=mybir.AxisListType.X, op=Alu.add,
                                        apply_absolute_value=True)
                bv = wt.tile([P, 1], f32, tag="bv")
                nc.vector.tensor_tensor(out=bv[:], in0=ba[:, 0:1],
                                        in1=ba[:, 1:2], op=Alu.add)
                bps = psum.tile([2, P], f32, tag="mm_ps")
                nc.tensor.matmul(bps[0:1, 0:1], lhsT=onesf[:], rhs=bv[:],
                                 start=True, stop=True)
                bt = wt.tile([1, 1], f32, tag=f"beta{li}")
                nc.scalar.activation(out=bt[:], in_=bps[0:1, 0:1],
                                     func=Act.Copy, scale=1.0 / (D * D))
                beta.append(bt)
                wq16 = wt.tile([P, 2, D], f16, tag="wq16")
                nc.vector.tensor_scalar(out=wq16[:], in0=wc[:], scalar1=0.0,
                                        scalar2=2.0, op0=Alu.is_gt,
                                        op1=Alu.mult)
                nc.vector.tensor_scalar(out=wq16[:], in0=wq16[:], scalar1=1.0,
                                        scalar2=None, op0=Alu.subtract)
                wqt = wt.tile([P, 2, D], f16, tag=f"wqT{li}")
                for a in range(2):
                    for k in range(2):
                        nc.sync.dma_start_transpose(
                            out=wqt[:, k, a * P:(a + 1) * P],
                            in_=wq16[:, a, k * P:(k + 1) * P])
                wqT.append(wqt)

            # per-layer stat tiles
            bnt = [wt.tile([P, T // 2, 6], f32, name=f"bnt{li}",
                           tag=f"bnt{li}") for li in range(3)]
            rmx16 = [wt.tile([P, T], i16, name=f"rmx{li}", tag=f"rmx{li}")
                     for li in range(1, 3)]
            rmx0 = wt.tile([P, T], f32, tag="rmx0")
            rmn0 = wt.tile([P, T], f32, tag="rmn0")

            def bn_group(li, src, g):
                for i in range(0, G, 2):
                    _in3d = src[:, i:i + 2, :].rearrange("p t d -> p d t")
                    nc.vector.add_instruction(mybir.InstBNStats(
                        name=nc.get_next_instruction_name(),
                        ins=[nc.vector.lower_ap(_in3d)],
                        outs=[nc.vector.lower_ap(
                            bnt[li][:, (g * G + i) // 2, :])]))

            # ---- L0 stats sweep over f32 x ----
            for g in range(NGRP):
                if g < NSTASH:
                    src = xhalf[:, g * G:(g + 1) * G, :]
                    nc.sync.dma_start(out=src, in_=x_d[
                        g * G * P:(g + 1) * G * P, :].rearrange(
                        "(t p) d -> p t d", p=P))
                else:
                    xg = f32stage.tile([P, G, D], f32, tag="stage")
                    nc.sync.dma_start(out=xg[:], in_=x_d[
                        g * G * P:(g + 1) * G * P, :].rearrange(
                        "(t p) d -> p t d", p=P))
                    src = xg
                bn_group(0, src, g)
                nc.vector.tensor_reduce(
                    out=rmx0[:, g * G:(g + 1) * G], in_=src,
                    axis=mybir.AxisListType.X, op=Alu.max)
                nc.vector.tensor_reduce(
                    out=rmn0[:, g * G:(g + 1) * G], in_=src,
                    axis=mybir.AxisListType.X, op=Alu.min)

            # ---------------- layers ----------------
            for li in range(3):
                last = li == 2
                # ---- per-row stat math ----
                mu = stats.tile([P, T], f32, tag="mu")
                var = stats.tile([P, T], f32, tag="var")
                nc.vector.tensor_copy(out=mu[:].rearrange(
                    "p (t two) -> p t two", two=2)[:, :, 0],
                    in_=bnt[li][:, :, 1])
                nc.vector.tensor_copy(out=mu[:].rearrange(
                    "p (t two) -> p t two", two=2)[:, :, 1],
                    in_=bnt[li][:, :, 4])
                nc.vector.tensor_scalar(out=var[:].rearrange(
                    "p (t two) -> p t two", two=2)[:, :, 0],
                    in0=bnt[li][:, :, 2], scalar1=1.0 / D, scalar2=None,
                    op0=Alu.mult)
                nc.vector.tensor_scalar(out=var[:].rearrange(
                    "p (t two) -> p t two", two=2)[:, :, 1],
                    in0=bnt[li][:, :, 5], scalar1=1.0 / D, scalar2=None,
                    op0=Alu.mult)
                rstd = stats.tile([P, T], f32, tag="rstd")
                nc.scalar.activation(out=rstd[:], in_=var[:], func=Act.Sqrt,
                                     bias=epst[:], scale=1.0)
                nc.vector.reciprocal(out=rstd[:], in_=rstd[:])
                a1 = stats.tile([P, T], f32, tag="a1")
                if li == 0:
                    nc.vector.tensor_tensor(out=a1[:], in0=rmx0[:], in1=mu[:],
                                            op=Alu.subtract)
                    a2 = stats.tile([P, T], f32, tag="a2")
                    nc.vector.tensor_tensor(out=a2[:], in0=mu[:], in1=rmn0[:],
                                            op=Alu.subtract)
                    nc.vector.tensor_tensor(out=a1[:], in0=a1[:], in1=a2[:],
                                            op=Alu.max)
                else:
                    rmxf = stats.tile([P, T], f32, tag="rmxf")
                    nc.vector.tensor_copy(out=rmxf[:], in_=rmx16[li - 1][:])
                    nc.vector.tensor_tensor(out=a1[:], in0=rmxf[:], in1=mu[:],
                                            op=Alu.subtract)
                    nc.vector.tensor_tensor(out=a1[:], in0=a1[:], in1=mu[:],
                                            op=Alu.max)
                nc.vector.tensor_tensor(out=a1[:], in0=a1[:], in1=rstd[:],
                                        op=Alu.mult)
                gl = stats.tile([P, 1], f32, tag="gl")
                nc.vector.tensor_reduce(out=gl[:], in_=a1[:],
                                        axis=mybir.AxisListType.X, op=Alu.max)
                nc.gpsimd.partition_all_reduce(gl[:], gl[:], channels=P,
                                               reduce_op=bass_isa.ReduceOp.max)
                nc.vector.tensor_scalar(out=gl[0:1, :], in0=gl[0:1, :],
                                        scalar1=1e-8, scalar2=None,
                                        op0=Alu.max)

                # ---- AllGather of local gamma candidate ----
                snd_sb = smallp.tile([1, 8], f32, tag="snd_sb")
                nc.gpsimd.tensor_copy(out=snd_sb[:],
                                      in_=gl[0:1, 0:1].broadcast_to((1, 8)))
                snd = dram.tile([1, 8], f32, tag="snd")
                rcv = dram.tile([1, 8 * NCORES], f32, tag="rcv")
                nc.sync.dma_start(out=snd[:], in_=snd_sb[:])
                nc.gpsimd.collective_compute(
                    "AllGather", Alu.bypass, ins=[snd[:].opt()],
                    outs=[rcv[:].opt()],
                    replica_groups=[list(range(NCORES))])
                g64 = smallp.tile([1, 8 * NCORES], f32, tag="g64")
                nc.sync.dma_start(out=g64[:], in_=rcv[:])
                gam = smallp.tile([1, 1], f32, tag="gam")
                nc.vector.tensor_reduce(out=gam[:], in_=g64[:],
                                        axis=mybir.AxisListType.X, op=Alu.max)
                gi = smallp.tile([1, 1], f32, tag="gi")
                nc.vector.reciprocal(out=gi[:], in_=gam[:])
                nc.vector.tensor_scalar(out=gi[:], in0=gi[:], scalar1=QB,
                                        scalar2=None, op0=Alu.mult)
                gbc = smallp.tile([P, 1], f32, tag="gbc")
                nc.gpsimd.partition_broadcast(gbc[:], gi[:])
                s1 = stats.tile([P, T], f32, tag="s1")
                nc.vector.tensor_scalar(out=s1[:], in0=rstd[:], scalar1=gbc[:],
                                        scalar2=None, op0=Alu.mult)
                tp = stats.tile([P, T], f32, tag="tp")
                nc.vector.tensor_tensor(out=tp[:], in0=mu[:], in1=s1[:],
                                        op=Alu.mult)
                nc.vector.tensor_scalar(out=tp[:], in0=tp[:], scalar1=-1.0,
                                        scalar2=OFF, op0=Alu.mult, op1=Alu.add)

                if last:
                    c3 = smallp.tile([1, 1], f32, tag="c3")
                    nc.vector.tensor_tensor(out=c3[:], in0=beta[li][:],
                                            in1=gam[:], op=Alu.mult)
                    nc.vector.tensor_scalar(out=c3[:], in0=c3[:],
                                            scalar1=1.0 / QB, scalar2=None,
                                            op0=Alu.mult)
                    c3bc = smallp.tile([P, 1], f32, tag="c3bc")
                    nc.gpsimd.partition_broadcast(c3bc[:], c3[:])

                # ---- sweep: quantize -> transpose -> matmul -> epilogue ----
                a, b = QSPLIT[0], QSPLIT[1]
                for g in range(NGRP):
                    if li == 0:
                        if g < NSTASH:
                            src_g = xhalf[:, g * G:(g + 1) * G, :]
                        else:
                            xg2 = f32stage.tile([P, G, D], f32, tag="stage")
                            nc.sync.dma_start(out=xg2[:], in_=x_d[
                                g * G * P:(g + 1) * G * P, :].rearrange(
                                "(t p) d -> p t d", p=P))
                            src_g = xg2
                    else:
                        src_g = arena[:, g * G:(g + 1) * G, :]
                    u16 = u16p.tile([P, G, D], f16, tag="u16")
                    for i in range(G):
                        t = g * G + i
                        if i < a:
                            nc.gpsimd.tensor_scalar(
                                out=u16[:, i, :], in0=src_g[:, i, :],
                                scalar1=s1[:, t:t + 1], scalar2=tp[:, t:t + 1],
                                op0=Alu.mult, op1=Alu.add)
                        elif i < b:
                            nc.scalar.activation(
                                out=u16[:, i, :], in_=src_g[:, i, :],
                                func=Act.Identity, bias=tp[:, t:t + 1],
                                scale=s1[:, t:t + 1])
                        else:
                            nc.vector.tensor_scalar(
                                out=u16[:, i, :], in0=src_g[:, i, :],
                                scalar1=s1[:, t:t + 1], scalar2=tp[:, t:t + 1],
                                op0=Alu.mult, op1=Alu.add)
                    nc.vector.tensor_scalar(
                        out=u16[:], in0=u16[:], scalar1=OFF, scalar2=None,
                        op0=Alu.subtract)
                    hT = u16tp.tile([P, 2 * G, P], f16, tag="u16T")
                    teng = nc.sync if g % 2 == 0 else nc.scalar
                    teng.dma_start_transpose(
                        out=hT[:], in_=u16[:].rearrange("p a d -> p (a d)"))
                    for h in range(2):
                        ps = psum.tile([P, G // 2, D], f32, tag="mm_ps")
                        for i in range(G // 2):
                            ii = h * (G // 2) + i
                            nc.tensor.matmul(ps[:, i, :],
                                             lhsT=hT[:, 2 * ii, :],
                                             rhs=wqT[li][:, 0, :],
                                             start=True, stop=False)
                            nc.tensor.matmul(ps[:, i, :],
                                             lhsT=hT[:, 2 * ii + 1, :],
                                             rhs=wqT[li][:, 1, :],
                                             start=False, stop=True)
                        if not last:
                            nc.scalar.activation(
                                out=arena[:, g * G + h * 4:g * G + h * 4 + 4, :],
                                in_=ps[:], func=Act.Relu, scale=1.0)
                        else:
                            og = f32stage.tile([P, G // 2, D], f32,
                                               tag="stage")
                            nc.scalar.activation(out=og[:], in_=ps[:],
                                                 func=Act.Copy, scale=c3bc[:])
                            nc.sync.dma_start(
                                out=out_d[(g * G + h * 4) * P:
                                          (g * G + h * 4 + 4) * P, :].rearrange(
                                    "(t p) d -> p t d", p=P), in_=og[:])
                    if not last:
                        nli = li + 1
                        ag = arena[:, g * G:(g + 1) * G, :]
                        bn_group(nli, ag, g)
                        # Pool engine cannot do int16 max; rowmax on DVE
                        nc.vector.tensor_reduce(
                            out=rmx16[li][:, g * G:(g + 1) * G], in_=ag,
                            axis=mybir.AxisListType.X, op=Alu.max)

    nc.compile()
    return nc


_NC_CACHE = None


def _get_nc():
    global _NC_CACHE
    if _NC_CACHE is None:
        _NC_CACHE = build_nc()
    return _NC_CACHE


def run(inputs, trace=False, **kw):
    nc = _get_nc()
    x = inputs["x"]
    in_maps = []
    for c in range(NCORES):
        in_maps.append({
            "x": np.ascontiguousarray(x[c * B_LOC:(c + 1) * B_LOC]),
            "W1": inputs["W1"], "W2": inputs["W2"], "W3": inputs["W3"],
        })
    res = run_bass_kernel_spmd(nc, in_maps, core_ids=list(range(NCORES)),
                               trace=trace, **kw)
    out = np.concatenate([r["out"] for r in res.results], axis=0)
    return out, res


def kernel(**inputs):
    out, _ = run(inputs)
    return out


# revision 19
# speedup vs baseline: 1.0355x; 1.0355x over previous
"""BitNet 3-layer MLP (B=131072, D=256) on 8 TRN2 NeuronCores, data-parallel.

v4 design. Numerics identical to the f32 reference path (rel ~4.5e-3):
x consumed at f32, inter-layer activations exact int16.

Schedule (all phases pipelined per group of 8 row-tiles):
  L0-A   x DMA (half stashed f32, half re-read later) -> BNStats + rowmax on
         DVE, rowmin via gpsimd tensor_tensor min-tree (DVE finishes).
  gamma  per-row math -> [P,1] -> gpsimd partition_all_reduce -> 32B
         AllGather (warmed up by a dummy AllGather at t=0) -> s1/tp.
  sweep  quantize u16 = x*s1 + tp + 1536 (rounds to int in f16; tiles split
         gpsimd/ACT/DVE) -> OFF subtract (DVE 4x f16) -> xbar transpose
         (alternating sync/scalar HWDGE rings to halve the FIFO serial cost)
         -> 2 matmuls per tile (PE, fp16 exact) -> epilogue per half-group
         (ACT relu -> arena i16) -> fused next-layer stats: BNStats (DVE) +
         rowmax (even groups DVE reduce, odd groups gpsimd min... max-tree).
  L2     epilogue scales by beta*gamma/127 (ACT) and DMAs out.

Known-bad paths avoided: bass_isa tensor_tensor_reduce crashes the runtime;
f16 x or f16 h costs ~1e-2 rel err (gamma is extremely sensitive to the
f16 rounding of the argmax element).
"""
import os
import numpy as np
from contextlib import ExitStack

from concourse import bass, tile, mybir
from concourse import bacc
from concourse.bass_utils import run_bass_kernel_spmd
from concourse import bass_isa

P = 128
D = 256
NCORES = 8
B = 131072
B_LOC = B // NCORES          # 16384
T = B_LOC // P               # 128 tiles
G = 8                        # tiles per group
NGRP = T // G                # 16 groups
NSTASH = 6                   # groups of x kept resident in f32
OFF = 1536.0                 # fp16 rounding offset
LN_EPS = 1e-5
QB = 127.0

f32 = mybir.dt.float32
f16 = mybir.dt.float16
i16 = mybir.dt.int16
Alu = mybir.AluOpType
Act = mybir.ActivationFunctionType

NOWARM = os.environ.get("KNOWARM") == "1"
# per-group quantize engine split: tiles [0,a) gpsimd, [a,b) ACT, [b,8) DVE
QSPLIT = [int(c) for c in os.environ.get("KQSPLIT", "45")]


def build_nc():
    nc = bacc.Bacc("TRN2", target_bir_lowering=False, debug=False,
                   num_devices=NCORES)

    x_d = nc.dram_tensor("x", [B_LOC, D], f32, kind="ExternalInput")
    w_d = [nc.dram_tensor(f"W{i+1}", [D, D], f32, kind="ExternalInput")
           for i in range(3)]
    out_d = nc.dram_tensor("out", [B_LOC, D], f32, kind="ExternalOutput")

    with tile.TileContext(nc) as tc:
        with ExitStack() as ctx:
            wt = ctx.enter_context(tc.tile_pool(name="wt", bufs=1))
            stats = ctx.enter_context(tc.tile_pool(name="stats", bufs=2))
            trp = ctx.enter_context(tc.tile_pool(name="trp", bufs=2))
            f32stage = ctx.enter_context(tc.tile_pool(name="f32stage", bufs=3))
            u16p = ctx.enter_context(tc.tile_pool(name="u16p", bufs=2))
            u16tp = ctx.enter_context(tc.tile_pool(name="u16tp", bufs=2))
            smallp = ctx.enter_context(tc.tile_pool(name="smallp", bufs=2))
            psum = ctx.enter_context(tc.tile_pool(name="psum", bufs=4,
                                                  space="PSUM"))
            dram = ctx.enter_context(tc.tile_pool(name="dram", bufs=2,
                                                  space="DRAM"))

            arena = wt.tile([P, T, D], i16)
            xhalf = wt.tile([P, NSTASH * G, D], f32)

            # ---------------- constants ----------------
            onesf = wt.tile([P, 1], f32)
            nc.vector.memset(onesf[:], 1.0)
            repl = wt.tile([1, P], f32)
            nc.vector.memset(repl[:], 1.0)
            epst = wt.tile([P, 1], f32)
            nc.vector.memset(epst[:], LN_EPS)

            # ---------------- warmup AllGather ----------------
            if not NOWARM:
                wsnd_sb = smallp.tile([1, 8], f32, tag="wsnd_sb")
                nc.gpsimd.memset(wsnd_sb[:], 1.0)
                wsnd = dram.tile([1, 8], f32, tag="snd")
                wrcv = dram.tile([1, 8 * NCORES], f32, tag="rcv")
                nc.sync.dma_start(out=wsnd[:], in_=wsnd_sb[:])
                nc.gpsimd.collective_compute(
                    "AllGather", Alu.bypass, ins=[wsnd[:].opt()],
                    outs=[wrcv[:].opt()],
                    replica_groups=[list(range(NCORES))])
                # no result fetch: a sync-queue DMA here would sit at the
                # head of the HWDGE FIFO and block the x loads ~50us

            # ---------------- weight prep ----------------
            wqT = []     # [128, 2, 256] fp16: wqT[k_in_band, band, j]
            beta = []    # [1, 1] f32
            for li in range(3):
                wf = wt.tile([P, 2, D], f32, tag="wf")
                nc.sync.dma_start(out=wf[:], in_=w_d[li][:].rearrange(
                    "(a p) d -> p a d", p=P))
                rs = wt.tile([P, 2], f32, tag="rs")
                nc.vector.tensor_reduce(out=rs[:], in_=wf[:],
                                        axis=mybir.AxisListType.X, op=Alu.add)
                rv = wt.tile([P, 1], f32, tag="rv")
                nc.vector.tensor_tensor(out=rv[:], in0=rs[:, 0:1],
                                        in1=rs[:, 1:2], op=Alu.add)
                aps = psum.tile([2, P], f32, tag="mm_ps")
                nc.tensor.matmul(aps[0:1, 0:1], lhsT=onesf[:], rhs=rv[:],
                                 start=True, stop=True)
                alpha = wt.tile([1, 1], f32, tag="alpha")
                nc.scalar.activation(out=alpha[:], in_=aps[0:1, 0:1],
                                     func=Act.Copy, scale=1.0 / (D * D))
                abc_ps = psum.tile([P, 1], f32, tag="mm_ps")
                nc.tensor.matmul(abc_ps[:], lhsT=repl[:], rhs=alpha[:],
                                 start=True, stop=True)
                abc = wt.tile([P, 1], f32, tag="abc")
                nc.vector.tensor_copy(out=abc[:], in_=abc_ps[:])
                wc = wt.tile([P, 2, D], f32, tag="wc")
                nc.vector.tensor_scalar(out=wc[:], in0=wf[:], scalar1=abc[:],
                                        scalar2=None, op0=Alu.subtract)
                ba = wt.tile([P, 2], f32, tag="ba")
                nc.vector.tensor_reduce(out=ba[:], in_=wc[:],
                                        axis=mybir.AxisListType.X, op=Alu.add,
                                        apply_absolute_value=True)
                bv = wt.tile([P, 1], f32, tag="bv")
                nc.vector.tensor_tensor(out=bv[:], in0=ba[:, 0:1],
                                        in1=ba[:, 1:2], op=Alu.add)
                bps = psum.tile([2, P], f32, tag="mm_ps")
                nc.tensor.matmul(bps[0:1, 0:1], lhsT=onesf[:], rhs=bv[:],
                                 start=True, stop=True)
                bt = wt.tile([1, 1], f32, tag=f"beta{li}")
                nc.scalar.activation(out=bt[:], in_=bps[0:1, 0:1],
                                     func=Act.Copy, scale=1.0 / (D * D))
                beta.append(bt)
                wq16 = wt.tile([P, 2, D], f16, tag="wq16")
                nc.vector.tensor_scalar(out=wq16[:], in0=wc[:], scalar1=0.0,
                                        scalar2=2.0, op0=Alu.is_gt,
                                        op1=Alu.mult)
                nc.vector.tensor_scalar(out=wq16[:], in0=wq16[:], scalar1=1.0,
                                        scalar2=None, op0=Alu.subtract)
                wqt = wt.tile([P, 2, D], f16, tag=f"wqT{li}")
                for a in range(2):
                    for k in range(2):
                        nc.sync.dma_start_transpose(
                            out=wqt[:, k, a * P:(a + 1) * P],
                            in_=wq16[:, a, k * P:(k + 1) * P])
                wqT.append(wqt)

            # per-layer stat tiles
            bnt = [wt.tile([P, T // 2, 6], f32, name=f"bnt{li}",
                           tag=f"bnt{li}") for li in range(3)]
            rmx16 = [wt.tile([P, T], i16, name=f"rmx{li}", tag=f"rmx{li}")
                     for li in range(1, 3)]
            rmx0 = wt.tile([P, T], f32, tag="rmx0")
            rmn0 = wt.tile([P, T], f32, tag="rmn0")

            def bn_group(li, src, g):
                for i in range(0, G, 2):
                    _in3d = src[:, i:i + 2, :].rearrange("p t d -> p d t")
                    nc.vector.add_instruction(mybir.InstBNStats(
                        name=nc.get_next_instruction_name(),
                        ins=[nc.vector.lower_ap(_in3d)],
                        outs=[nc.vector.lower_ap(
                            bnt[li][:, (g * G + i) // 2, :])]))

            # ---- L0 stats sweep over f32 x ----
            for g in range(NGRP):
                if g < NSTASH:
                    src = xhalf[:, g * G:(g + 1) * G, :]
                    nc.sync.dma_start(out=src, in_=x_d[
                        g * G * P:(g + 1) * G * P, :].rearrange(
                        "(t p) d -> p t d", p=P))
                else:
                    xg = f32stage.tile([P, G, D], f32, tag="stage")
                    nc.sync.dma_start(out=xg[:], in_=x_d[
                        g * G * P:(g + 1) * G * P, :].rearrange(
                        "(t p) d -> p t d", p=P))
                    src = xg
                bn_group(0, src, g)
                nc.vector.tensor_reduce(
                    out=rmx0[:, g * G:(g + 1) * G], in_=src,
                    axis=mybir.AxisListType.X, op=Alu.max)
                nc.vector.tensor_reduce(
                    out=rmn0[:, g * G:(g + 1) * G], in_=src,
                    axis=mybir.AxisListType.X, op=Alu.min)

            # ---------------- layers ----------------
            for li in range(3):
                last = li == 2
                # ---- per-row stat math ----
                mu = stats.tile([P, T], f32, tag="mu")
                var = stats.tile([P, T], f32, tag="var")
                nc.vector.tensor_copy(out=mu[:].rearrange(
                    "p (t two) -> p t two", two=2)[:, :, 0],
                    in_=bnt[li][:, :, 1])
                nc.vector.tensor_copy(out=mu[:].rearrange(
                    "p (t two) -> p t two", two=2)[:, :, 1],
                    in_=bnt[li][:, :, 4])
                nc.vector.tensor_scalar(out=var[:].rearrange(
                    "p (t two) -> p t two", two=2)[:, :, 0],
                    in0=bnt[li][:, :, 2], scalar1=1.0 / D, scalar2=None,
                    op0=Alu.mult)
                nc.vector.tensor_scalar(out=var[:].rearrange(
                    "p (t two) -> p t two", two=2)[:, :, 1],
                    in0=bnt[li][:, :, 5], scalar1=1.0 / D, scalar2=None,
                    op0=Alu.mult)
                rstd = stats.tile([P, T], f32, tag="rstd")
                nc.scalar.activation(out=rstd[:], in_=var[:], func=Act.Sqrt,
                                     bias=epst[:], scale=1.0)
                nc.vector.reciprocal(out=rstd[:], in_=rstd[:])
                a1 = stats.tile([P, T], f32, tag="a1")
                if li == 0:
                    nc.vector.tensor_tensor(out=a1[:], in0=rmx0[:], in1=mu[:],
                                            op=Alu.subtract)
                    a2 = stats.tile([P, T], f32, tag="a2")
                    nc.vector.tensor_tensor(out=a2[:], in0=mu[:], in1=rmn0[:],
                                            op=Alu.subtract)
                    nc.vector.tensor_tensor(out=a1[:], in0=a1[:], in1=a2[:],
                                            op=Alu.max)
                else:
                    rmxf = stats.tile([P, T], f32, tag="rmxf")
                    nc.vector.tensor_copy(out=rmxf[:], in_=rmx16[li - 1][:])
                    nc.vector.tensor_tensor(out=a1[:], in0=rmxf[:], in1=mu[:],
                                            op=Alu.subtract)
                    nc.vector.tensor_tensor(out=a1[:], in0=a1[:], in1=mu[:],
                                            op=Alu.max)
                nc.vector.tensor_tensor(out=a1[:], in0=a1[:], in1=rstd[:],
                                        op=Alu.mult)
                gl = stats.tile([P, 1], f32, tag="gl")
                nc.vector.tensor_reduce(out=gl[:], in_=a1[:],
                                        axis=mybir.AxisListType.X, op=Alu.max)
                nc.gpsimd.partition_all_reduce(gl[:], gl[:], channels=P,
                                               reduce_op=bass_isa.ReduceOp.max)
                nc.vector.tensor_scalar(out=gl[0:1, :], in0=gl[0:1, :],
                                        scalar1=1e-8, scalar2=None,
                                        op0=Alu.max)

                # ---- AllGather of local gamma candidate ----
                snd_sb = smallp.tile([1, 8], f32, tag="snd_sb")
                nc.gpsimd.tensor_copy(out=snd_sb[:],
                                      in_=gl[0:1, 0:1].broadcast_to((1, 8)))
                snd = dram.tile([1, 8], f32, tag="snd")
                rcv = dram.tile([1, 8 * NCORES], f32, tag="rcv")
                nc.sync.dma_start(out=snd[:], in_=snd_sb[:])
                nc.gpsimd.collective_compute(
                    "AllGather", Alu.bypass, ins=[snd[:].opt()],
                    outs=[rcv[:].opt()],
                    replica_groups=[list(range(NCORES))])
                g64 = smallp.tile([1, 8 * NCORES], f32, tag="g64")
                nc.sync.dma_start(out=g64[:], in_=rcv[:])
                gam = smallp.tile([1, 1], f32, tag="gam")
                nc.vector.tensor_reduce(out=gam[:], in_=g64[:],
                                        axis=mybir.AxisListType.X, op=Alu.max)
                gi = smallp.tile([1, 1], f32, tag="gi")
                nc.vector.reciprocal(out=gi[:], in_=gam[:])
                nc.vector.tensor_scalar(out=gi[:], in0=gi[:], scalar1=QB,
                                        scalar2=None, op0=Alu.mult)
                gbc = smallp.tile([P, 1], f32, tag="gbc")
                nc.gpsimd.partition_broadcast(gbc[:], gi[:])
                s1 = stats.tile([P, T], f32, tag="s1")
                nc.vector.tensor_scalar(out=s1[:], in0=rstd[:], scalar1=gbc[:],
                                        scalar2=None, op0=Alu.mult)
                tp = stats.tile([P, T], f32, tag="tp")
                nc.vector.tensor_tensor(out=tp[:], in0=mu[:], in1=s1[:],
                                        op=Alu.mult)
                nc.vector.tensor_scalar(out=tp[:], in0=tp[:], scalar1=-1.0,
                                        scalar2=OFF, op0=Alu.mult, op1=Alu.add)

                if last:
                    c3 = smallp.tile([1, 1], f32, tag="c3")
                    nc.vector.tensor_tensor(out=c3[:], in0=beta[li][:],
                                            in1=gam[:], op=Alu.mult)
                    nc.vector.tensor_scalar(out=c3[:], in0=c3[:],
                                            scalar1=1.0 / QB, scalar2=None,
                                            op0=Alu.mult)
                    c3bc = smallp.tile([P, 1], f32, tag="c3bc")
                    nc.gpsimd.partition_broadcast(c3bc[:], c3[:])

                # ---- sweep: quantize -> transpose -> matmul -> epilogue ----
                a, b = QSPLIT[0], QSPLIT[1]
                for g in range(NGRP):
                    if li == 0:
                        if g < NSTASH:
                            src_g = xhalf[:, g * G:(g + 1) * G, :]
                        else:
                            xg2 = f32stage.tile([P, G, D], f32, tag="stage")
                            nc.sync.dma_start(out=xg2[:], in_=x_d[
                                g * G * P:(g + 1) * G * P, :].rearrange(
                                "(t p) d -> p t d", p=P))
                            src_g = xg2
                    else:
                        src_g = arena[:, g * G:(g + 1) * G, :]
                    u16 = u16p.tile([P, G, D], f16, tag="u16")
                    for i in range(G):
                        t = g * G + i
                        if i < a:
                            nc.gpsimd.tensor_scalar(
                                out=u16[:, i, :], in0=src_g[:, i, :],
                                scalar1=s1[:, t:t + 1], scalar2=tp[:, t:t + 1],
                                op0=Alu.mult, op1=Alu.add)
                        elif i < b:
                            nc.scalar.activation(
                                out=u16[:, i, :], in_=src_g[:, i, :],
                                func=Act.Identity, bias=tp[:, t:t + 1],
                                scale=s1[:, t:t + 1])
                        else:
                            nc.vector.tensor_scalar(
                                out=u16[:, i, :], in0=src_g[:, i, :],
                                scalar1=s1[:, t:t + 1], scalar2=tp[:, t:t + 1],
                                op0=Alu.mult, op1=Alu.add)
                    nc.vector.tensor_scalar(
                        out=u16[:], in0=u16[:], scalar1=OFF, scalar2=None,
                        op0=Alu.subtract)
                    hT = u16tp.tile([P, 2 * G, P], f16, tag="u16T")
                    nc.sync.dma_start_transpose(
                        out=hT[:], in_=u16[:].rearrange("p a d -> p (a d)"))
                    for h in range(2):
                        ps = psum.tile([P, G // 2, D], f32, tag="mm_ps")
                        for i in range(G // 2):
                            ii = h * (G // 2) + i
                            nc.tensor.matmul(ps[:, i, :],
                                             lhsT=hT[:, 2 * ii, :],
                                             rhs=wqT[li][:, 0, :],
                                             start=True, stop=False)
                            nc.tensor.matmul(ps[:, i, :],
                                             lhsT=hT[:, 2 * ii + 1, :],
                                             rhs=wqT[li][:, 1, :],
                                             start=False, stop=True)
                        if not last:
                            nc.scalar.activation(
                                out=arena[:, g * G + h * 4:g * G + h * 4 + 4, :],
                                in_=ps[:], func=Act.Relu, scale=1.0)
                        else:
                            og = f32stage.tile([P, G // 2, D], f32,
                                               tag="stage")
                            nc.scalar.activation(out=og[:], in_=ps[:],
                                                 func=Act.Copy, scale=c3bc[:])
                            nc.sync.dma_start(
                                out=out_d[(g * G + h * 4) * P:
                                          (g * G + h * 4 + 4) * P, :].rearrange(
                                    "(t p) d -> p t d", p=P), in_=og[:])
                    if not last:
                        nli = li + 1
                        ag = arena[:, g * G:(g + 1) * G, :]
                        bn_group(nli, ag, g)
                        # Pool engine cannot do int16 max; rowmax on DVE
                        nc.vector.tensor_reduce(
                            out=rmx16[li][:, g * G:(g + 1) * G], in_=ag,
                            axis=mybir.AxisListType.X, op=Alu.max)

    nc.compile()
    return nc


_NC_CACHE = None


def _get_nc():
    global _NC_CACHE
    if _NC_CACHE is None:
        _NC_CACHE = build_nc()
    return _NC_CACHE


def run(inputs, trace=False, **kw):
    nc = _get_nc()
    x = inputs["x"]
    in_maps = []
    for c in range(NCORES):
        in_maps.append({
            "x": np.ascontiguousarray(x[c * B_LOC:(c + 1) * B_LOC]),
            "W1": inputs["W1"], "W2": inputs["W2"], "W3": inputs["W3"],
        })
    res = run_bass_kernel_spmd(nc, in_maps, core_ids=list(range(NCORES)),
                               trace=trace, **kw)
    out = np.concatenate([r["out"] for r in res.results], axis=0)
    return out, res


def kernel(**inputs):
    out, _ = run(inputs)
    return out


# revision 20
# speedup vs baseline: 1.1283x; 1.0897x over previous
"""BitNet 3-layer MLP (B=131072, D=256) on 8 TRN2 NeuronCores, data-parallel.

v4 design. Numerics identical to the f32 reference path (rel ~4.5e-3):
x consumed at f32, inter-layer activations exact int16.

Schedule (all phases pipelined per group of 8 row-tiles):
  L0-A   x DMA (half stashed f32, half re-read later) -> BNStats + rowmax on
         DVE, rowmin via gpsimd tensor_tensor min-tree (DVE finishes).
  gamma  per-row math -> [P,1] -> gpsimd partition_all_reduce -> 32B
         AllGather (warmed up by a dummy AllGather at t=0) -> s1/tp.
  sweep  quantize u16 = x*s1 + tp + 1536 (rounds to int in f16; tiles split
         gpsimd/ACT/DVE) -> OFF subtract (DVE 4x f16) -> xbar transpose
         (alternating sync/scalar HWDGE rings to halve the FIFO serial cost)
         -> 2 matmuls per tile (PE, fp16 exact) -> epilogue per half-group
         (ACT relu -> arena i16) -> fused next-layer stats: BNStats (DVE) +
         rowmax (even groups DVE reduce, odd groups gpsimd min... max-tree).
  L2     epilogue scales by beta*gamma/127 (ACT) and DMAs out.

Known-bad paths avoided: bass_isa tensor_tensor_reduce crashes the runtime;
f16 x or f16 h costs ~1e-2 rel err (gamma is extremely sensitive to the
f16 rounding of the argmax element).
"""
import os
import numpy as np
from contextlib import ExitStack

from concourse import bass, tile, mybir
from concourse import bacc
from concourse.bass_utils import run_bass_kernel_spmd
from concourse import bass_isa

P = 128
D = 256
NCORES = 8
B = 131072
B_LOC = B // NCORES          # 16384
T = B_LOC // P               # 128 tiles
G = 8                        # tiles per group
NGRP = T // G                # 16 groups
NSTASH = 6                   # groups of x kept resident in f32
OFF = 1536.0                 # fp16 rounding offset
LN_EPS = 1e-5
QB = 127.0

f32 = mybir.dt.float32
f16 = mybir.dt.float16
i16 = mybir.dt.int16
Alu = mybir.AluOpType
Act = mybir.ActivationFunctionType

NOWARM = os.environ.get("KNOWARM") == "1"
# per-group quantize engine split: tiles [0,a) gpsimd, [a,b) ACT, [b,8) DVE
QSPLIT = [int(c) for c in os.environ.get("KQSPLIT", "57")]


def build_nc():
    nc = bacc.Bacc("TRN2", target_bir_lowering=False, debug=False,
                   num_devices=NCORES)

    x_d = nc.dram_tensor("x", [B_LOC, D], f32, kind="ExternalInput")
    w_d = [nc.dram_tensor(f"W{i+1}", [D, D], f32, kind="ExternalInput")
           for i in range(3)]
    out_d = nc.dram_tensor("out", [B_LOC, D], f32, kind="ExternalOutput")

    with tile.TileContext(nc) as tc:
        with ExitStack() as ctx:
            wt = ctx.enter_context(tc.tile_pool(name="wt", bufs=1))
            stats = ctx.enter_context(tc.tile_pool(name="stats", bufs=2))
            trp = ctx.enter_context(tc.tile_pool(name="trp", bufs=2))
            f32stage = ctx.enter_context(tc.tile_pool(name="f32stage", bufs=3))
            u16p = ctx.enter_context(tc.tile_pool(name="u16p", bufs=2))
            u16tp = ctx.enter_context(tc.tile_pool(name="u16tp", bufs=2))
            smallp = ctx.enter_context(tc.tile_pool(name="smallp", bufs=2))
            psum = ctx.enter_context(tc.tile_pool(name="psum", bufs=4,
                                                  space="PSUM"))
            dram = ctx.enter_context(tc.tile_pool(name="dram", bufs=2,
                                                  space="DRAM"))

            arena = wt.tile([P, T, D], i16)
            xhalf = wt.tile([P, NSTASH * G, D], f32)

            # ---------------- constants ----------------
            onesf = wt.tile([P, 1], f32)
            nc.vector.memset(onesf[:], 1.0)
            repl = wt.tile([1, P], f32)
            nc.vector.memset(repl[:], 1.0)
            epst = wt.tile([P, 1], f32)
            nc.vector.memset(epst[:], LN_EPS)

            # per-layer stat tiles
            bnt = [wt.tile([P, T // 2, 6], f32, name=f"bnt{li}",
                           tag=f"bnt{li}") for li in range(3)]
            rmx16 = [wt.tile([P, T], i16, name=f"rmx{li}", tag=f"rmx{li}")
                     for li in range(1, 3)]
            rmx0 = wt.tile([P, T], f32, tag="rmx0")
            rmn0 = wt.tile([P, T], f32, tag="rmn0")

            def bn_group(li, src, g):
                for i in range(0, G, 2):
                    _in3d = src[:, i:i + 2, :].rearrange("p t d -> p d t")
                    nc.vector.add_instruction(mybir.InstBNStats(
                        name=nc.get_next_instruction_name(),
                        ins=[nc.vector.lower_ap(_in3d)],
                        outs=[nc.vector.lower_ap(
                            bnt[li][:, (g * G + i) // 2, :])]))

            # ---- L0 stats sweep over f32 x ----
            for g in range(NGRP):
                if g < NSTASH:
                    src = xhalf[:, g * G:(g + 1) * G, :]
                    nc.sync.dma_start(out=src, in_=x_d[
                        g * G * P:(g + 1) * G * P, :].rearrange(
                        "(t p) d -> p t d", p=P))
                else:
                    xg = f32stage.tile([P, G, D], f32, tag="stage")
                    nc.sync.dma_start(out=xg[:], in_=x_d[
                        g * G * P:(g + 1) * G * P, :].rearrange(
                        "(t p) d -> p t d", p=P))
                    src = xg
                bn_group(0, src, g)
                nc.vector.tensor_reduce(
                    out=rmx0[:, g * G:(g + 1) * G], in_=src,
                    axis=mybir.AxisListType.X, op=Alu.max)
                nc.vector.tensor_reduce(
                    out=rmn0[:, g * G:(g + 1) * G], in_=src,
                    axis=mybir.AxisListType.X, op=Alu.min)

            # ---------------- warmup AllGather ----------------
            if not NOWARM:
                wsnd_sb = smallp.tile([1, 8], f32, tag="wsnd_sb")
                nc.gpsimd.memset(wsnd_sb[:], 1.0)
                wsnd = dram.tile([1, 8], f32, tag="snd")
                wrcv = dram.tile([1, 8 * NCORES], f32, tag="rcv")
                nc.gpsimd.dma_start(out=wsnd[:], in_=wsnd_sb[:])
                nc.gpsimd.collective_compute(
                    "AllGather", Alu.bypass, ins=[wsnd[:].opt()],
                    outs=[wrcv[:].opt()],
                    replica_groups=[list(range(NCORES))])
                # no result fetch: a sync-queue DMA here would sit at the
                # head of the HWDGE FIFO and block the x loads ~50us

            # ---------------- weight prep ----------------
            wqT = []     # [128, 2, 256] fp16: wqT[k_in_band, band, j]
            beta = []    # [1, 1] f32
            for li in range(3):
                wf = wt.tile([P, 2, D], f32, tag="wf")
                nc.sync.dma_start(out=wf[:], in_=w_d[li][:].rearrange(
                    "(a p) d -> p a d", p=P))
                rs = wt.tile([P, 2], f32, tag="rs")
                nc.vector.tensor_reduce(out=rs[:], in_=wf[:],
                                        axis=mybir.AxisListType.X, op=Alu.add)
                rv = wt.tile([P, 1], f32, tag="rv")
                nc.vector.tensor_tensor(out=rv[:], in0=rs[:, 0:1],
                                        in1=rs[:, 1:2], op=Alu.add)
                aps = psum.tile([2, P], f32, tag="mm_ps")
                nc.tensor.matmul(aps[0:1, 0:1], lhsT=onesf[:], rhs=rv[:],
                                 start=True, stop=True)
                alpha = wt.tile([1, 1], f32, tag="alpha")
                nc.scalar.activation(out=alpha[:], in_=aps[0:1, 0:1],
                                     func=Act.Copy, scale=1.0 / (D * D))
                abc_ps = psum.tile([P, 1], f32, tag="mm_ps")
                nc.tensor.matmul(abc_ps[:], lhsT=repl[:], rhs=alpha[:],
                                 start=True, stop=True)
                abc = wt.tile([P, 1], f32, tag="abc")
                nc.vector.tensor_copy(out=abc[:], in_=abc_ps[:])
                wc = wt.tile([P, 2, D], f32, tag="wc")
                nc.vector.tensor_scalar(out=wc[:], in0=wf[:], scalar1=abc[:],
                                        scalar2=None, op0=Alu.subtract)
                ba = wt.tile([P, 2], f32, tag="ba")
                nc.vector.tensor_reduce(out=ba[:], in_=wc[:],
                                        axis=mybir.AxisListType.X, op=Alu.add,
                                        apply_absolute_value=True)
                bv = wt.tile([P, 1], f32, tag="bv")
                nc.vector.tensor_tensor(out=bv[:], in0=ba[:, 0:1],
                                        in1=ba[:, 1:2], op=Alu.add)
                bps = psum.tile([2, P], f32, tag="mm_ps")
                nc.tensor.matmul(bps[0:1, 0:1], lhsT=onesf[:], rhs=bv[:],
                                 start=True, stop=True)
                bt = wt.tile([1, 1], f32, tag=f"beta{li}")
                nc.scalar.activation(out=bt[:], in_=bps[0:1, 0:1],
                                     func=Act.Copy, scale=1.0 / (D * D))
                beta.append(bt)
                wq16 = wt.tile([P, 2, D], f16, tag="wq16")
                nc.vector.tensor_scalar(out=wq16[:], in0=wc[:], scalar1=0.0,
                                        scalar2=2.0, op0=Alu.is_gt,
                                        op1=Alu.mult)
                nc.vector.tensor_scalar(out=wq16[:], in0=wq16[:], scalar1=1.0,
                                        scalar2=None, op0=Alu.subtract)
                wqt = wt.tile([P, 2, D], f16, tag=f"wqT{li}")
                for a in range(2):
                    for k in range(2):
                        nc.sync.dma_start_transpose(
                            out=wqt[:, k, a * P:(a + 1) * P],
                            in_=wq16[:, a, k * P:(k + 1) * P])
                wqT.append(wqt)

            # ---------------- layers ----------------
            for li in range(3):
                last = li == 2
                # ---- per-row stat math ----
                mu = stats.tile([P, T], f32, tag="mu")
                var = stats.tile([P, T], f32, tag="var")
                nc.vector.tensor_copy(out=mu[:].rearrange(
                    "p (t two) -> p t two", two=2)[:, :, 0],
                    in_=bnt[li][:, :, 1])
                nc.vector.tensor_copy(out=mu[:].rearrange(
                    "p (t two) -> p t two", two=2)[:, :, 1],
                    in_=bnt[li][:, :, 4])
                nc.vector.tensor_scalar(out=var[:].rearrange(
                    "p (t two) -> p t two", two=2)[:, :, 0],
                    in0=bnt[li][:, :, 2], scalar1=1.0 / D, scalar2=None,
                    op0=Alu.mult)
                nc.vector.tensor_scalar(out=var[:].rearrange(
                    "p (t two) -> p t two", two=2)[:, :, 1],
                    in0=bnt[li][:, :, 5], scalar1=1.0 / D, scalar2=None,
                    op0=Alu.mult)
                rstd = stats.tile([P, T], f32, tag="rstd")
                nc.scalar.activation(out=rstd[:], in_=var[:], func=Act.Sqrt,
                                     bias=epst[:], scale=1.0)
                nc.vector.reciprocal(out=rstd[:], in_=rstd[:])
                a1 = stats.tile([P, T], f32, tag="a1")
                if li == 0:
                    nc.vector.tensor_tensor(out=a1[:], in0=rmx0[:], in1=mu[:],
                                            op=Alu.subtract)
                    a2 = stats.tile([P, T], f32, tag="a2")
                    nc.vector.tensor_tensor(out=a2[:], in0=mu[:], in1=rmn0[:],
                                            op=Alu.subtract)
                    nc.vector.tensor_tensor(out=a1[:], in0=a1[:], in1=a2[:],
                                            op=Alu.max)
                else:
                    rmxf = stats.tile([P, T], f32, tag="rmxf")
                    nc.vector.tensor_copy(out=rmxf[:], in_=rmx16[li - 1][:])
                    nc.vector.tensor_tensor(out=a1[:], in0=rmxf[:], in1=mu[:],
                                            op=Alu.subtract)
                    nc.vector.tensor_tensor(out=a1[:], in0=a1[:], in1=mu[:],
                                            op=Alu.max)
                nc.vector.tensor_tensor(out=a1[:], in0=a1[:], in1=rstd[:],
                                        op=Alu.mult)
                gl = stats.tile([P, 1], f32, tag="gl")
                nc.vector.tensor_reduce(out=gl[:], in_=a1[:],
                                        axis=mybir.AxisListType.X, op=Alu.max)
                nc.gpsimd.partition_all_reduce(gl[:], gl[:], channels=P,
                                               reduce_op=bass_isa.ReduceOp.max)
                nc.vector.tensor_scalar(out=gl[0:1, :], in0=gl[0:1, :],
                                        scalar1=1e-8, scalar2=None,
                                        op0=Alu.max)

                # ---- AllGather of local gamma candidate ----
                snd_sb = smallp.tile([1, 8], f32, tag="snd_sb")
                nc.gpsimd.tensor_copy(out=snd_sb[:],
                                      in_=gl[0:1, 0:1].broadcast_to((1, 8)))
                snd = dram.tile([1, 8], f32, tag="snd")
                rcv = dram.tile([1, 8 * NCORES], f32, tag="rcv")
                nc.sync.dma_start(out=snd[:], in_=snd_sb[:])
                nc.gpsimd.collective_compute(
                    "AllGather", Alu.bypass, ins=[snd[:].opt()],
                    outs=[rcv[:].opt()],
                    replica_groups=[list(range(NCORES))])
                g64 = smallp.tile([1, 8 * NCORES], f32, tag="g64")
                nc.sync.dma_start(out=g64[:], in_=rcv[:])
                gam = smallp.tile([1, 1], f32, tag="gam")
                nc.vector.tensor_reduce(out=gam[:], in_=g64[:],
                                        axis=mybir.AxisListType.X, op=Alu.max)
                gi = smallp.tile([1, 1], f32, tag="gi")
                nc.vector.reciprocal(out=gi[:], in_=gam[:])
                nc.vector.tensor_scalar(out=gi[:], in0=gi[:], scalar1=QB,
                                        scalar2=None, op0=Alu.mult)
                gbc = smallp.tile([P, 1], f32, tag="gbc")
                nc.gpsimd.partition_broadcast(gbc[:], gi[:])
                s1 = stats.tile([P, T], f32, tag="s1")
                nc.vector.tensor_scalar(out=s1[:], in0=rstd[:], scalar1=gbc[:],
                                        scalar2=None, op0=Alu.mult)
                tp = stats.tile([P, T], f32, tag="tp")
                nc.vector.tensor_tensor(out=tp[:], in0=mu[:], in1=s1[:],
                                        op=Alu.mult)
                nc.vector.tensor_scalar(out=tp[:], in0=tp[:], scalar1=-1.0,
                                        scalar2=OFF, op0=Alu.mult, op1=Alu.add)

                if last:
                    c3 = smallp.tile([1, 1], f32, tag="c3")
                    nc.vector.tensor_tensor(out=c3[:], in0=beta[li][:],
                                            in1=gam[:], op=Alu.mult)
                    nc.vector.tensor_scalar(out=c3[:], in0=c3[:],
                                            scalar1=1.0 / QB, scalar2=None,
                                            op0=Alu.mult)
                    c3bc = smallp.tile([P, 1], f32, tag="c3bc")
                    nc.gpsimd.partition_broadcast(c3bc[:], c3[:])

                # ---- sweep: quantize -> transpose -> matmul -> epilogue ----
                a, b = QSPLIT[0], QSPLIT[1]
                for g in range(NGRP):
                    if li == 0:
                        if g < NSTASH:
                            src_g = xhalf[:, g * G:(g + 1) * G, :]
                        else:
                            xg2 = f32stage.tile([P, G, D], f32, tag="stage")
                            nc.sync.dma_start(out=xg2[:], in_=x_d[
                                g * G * P:(g + 1) * G * P, :].rearrange(
                                "(t p) d -> p t d", p=P))
                            src_g = xg2
                    else:
                        src_g = arena[:, g * G:(g + 1) * G, :]
                    u16 = u16p.tile([P, G, D], f16, tag="u16")
                    for i in range(G):
                        t = g * G + i
                        if i < a:
                            nc.gpsimd.tensor_scalar(
                                out=u16[:, i, :], in0=src_g[:, i, :],
                                scalar1=s1[:, t:t + 1], scalar2=tp[:, t:t + 1],
                                op0=Alu.mult, op1=Alu.add)
                        elif i < b:
                            nc.scalar.activation(
                                out=u16[:, i, :], in_=src_g[:, i, :],
                                func=Act.Identity, bias=tp[:, t:t + 1],
                                scale=s1[:, t:t + 1])
                        else:
                            nc.vector.tensor_scalar(
                                out=u16[:, i, :], in0=src_g[:, i, :],
                                scalar1=s1[:, t:t + 1], scalar2=tp[:, t:t + 1],
                                op0=Alu.mult, op1=Alu.add)
                    nc.vector.tensor_scalar(
                        out=u16[:], in0=u16[:], scalar1=OFF, scalar2=None,
                        op0=Alu.subtract)
                    hT = u16tp.tile([P, 2 * G, P], f16, tag="u16T")
                    nc.sync.dma_start_transpose(
                        out=hT[:], in_=u16[:].rearrange("p a d -> p (a d)"))
                    for h in range(2):
                        ps = psum.tile([P, G // 2, D], f32, tag="mm_ps")
                        for i in range(G // 2):
                            ii = h * (G // 2) + i
                            nc.tensor.matmul(ps[:, i, :],
                                             lhsT=hT[:, 2 * ii, :],
                                             rhs=wqT[li][:, 0, :],
                                             start=True, stop=False)
                            nc.tensor.matmul(ps[:, i, :],
                                             lhsT=hT[:, 2 * ii + 1, :],
                                             rhs=wqT[li][:, 1, :],
                                             start=False, stop=True)
                        if not last:
                            nc.scalar.activation(
                                out=arena[:, g * G + h * 4:g * G + h * 4 + 4, :],
                                in_=ps[:], func=Act.Relu, scale=1.0)
                        else:
                            og = f32stage.tile([P, G // 2, D], f32,
                                               tag="stage")
                            nc.scalar.activation(out=og[:], in_=ps[:],
                                                 func=Act.Copy, scale=c3bc[:])
                            nc.sync.dma_start(
                                out=out_d[(g * G + h * 4) * P:
                                          (g * G + h * 4 + 4) * P, :].rearrange(
                                    "(t p) d -> p t d", p=P), in_=og[:])
                    if not last:
                        nli = li + 1
                        ag = arena[:, g * G:(g + 1) * G, :]
                        bn_group(nli, ag, g)
                        # Pool engine cannot do int16 max; rowmax on DVE
                        nc.vector.tensor_reduce(
                            out=rmx16[li][:, g * G:(g + 1) * G], in_=ag,
                            axis=mybir.AxisListType.X, op=Alu.max)

    nc.compile()
    return nc


_NC_CACHE = None


def _get_nc():
    global _NC_CACHE
    if _NC_CACHE is None:
        _NC_CACHE = build_nc()
    return _NC_CACHE


def run(inputs, trace=False, **kw):
    nc = _get_nc()
    x = inputs["x"]
    in_maps = []
    for c in range(NCORES):
        in_maps.append({
            "x": np.ascontiguousarray(x[c * B_LOC:(c + 1) * B_LOC]),
            "W1": inputs["W1"], "W2": inputs["W2"], "W3": inputs["W3"],
        })
    res = run_bass_kernel_spmd(nc, in_maps, core_ids=list(range(NCORES)),
                               trace=trace, **kw)
    out = np.concatenate([r["out"] for r in res.results], axis=0)
    return out, res


def kernel(**inputs):
    out, _ = run(inputs)
    return out
